# revision 13
# baseline (speedup 1.0000x reference)
"""GCN encoder (conv->BN->ReLU->2 conv heads) on 8 TRN2 NeuronCores.

Sharding: nodes (dst) split 8 ways. Each layer computes
agg[d] = dis_d * (sum_{e: dst=d} hs[src_e] + hs[d]) with hs = (h@W)*dis.
Per layer each core transforms its shard, publishes bf16 table rows via 4
quarter-wise AllGathers (overlapped with transform), then aggregates its dst
shard RANGE-MAJOR into an SBUF f32 accumulator: for each source quarter r,
merged gather calls (4 dst pairs each, int16 indices, 4 SWDGE queues round-
robin) pull per-edge source rows; one-hot P tiles (fp8, 256 dst cols) are
built on-chip from a compact per-slot dst-index table (DVE is_equal for most
tiles, scalar-engine relu(1-(x-d)^2) for the tail to balance engines); PE
accumulates z[feat, dst] += M_t^T @ P_t in PSUM per (pair, r); DVE folds each
round into z_acc. Self-loop terms enter via identity matmuls on the core's
own staged rows during round 0. Range-major order lets round r start as soon
as AllGather(r) lands, hiding the collective latency at both layer starts.
Evictions fuse the dis scale and BN stats; BN+ReLU for table2 is a scalar
activation chain around a PE transpose; head matmuls consume z [feat, dst]
directly with biases folded into the PSUM->SBUF activation copy.
"""

import os
import sys

sys.path.insert(0, "/opt/trn_rl_repo")

import numpy as np
import ml_dtypes

from concourse import bacc, bass, mybir, tile
from concourse.bass_utils import run_bass_kernel_spmd

bf16 = ml_dtypes.bfloat16

N = 100000
E = 1600000
IN = 256
HID = 128
OUT = 64
BN_EPS = 1e-5
NCORES = 8
SH = N // NCORES            # 12500 nodes per core
NB = (SH + 127) // 128      # 98 dst blocks (last has 84 nodes)
NPAIR = NB // 2             # 49 dst pairs (256 cols each)
NRANGE = 4
GP = 4                      # pairs per gather call
NCG = (NPAIR + GP - 1) // GP    # 13 call groups (last has 1 pair)
SCT = 6                     # tail tiles per call generated on scalar engine
# quarter split of each core's shard (block-aligned starts; q3 short)
QOFF = [0, 3200, 6400, 9600, SH]
QROWS = [QOFF[i + 1] - QOFF[i] for i in range(4)]      # 3200,3200,3200,2900
QBLK0 = [0, 25, 50, 75, NB]                            # block ranges per quarter
RSTART = [0]
for q in range(4):
    RSTART.append(RSTART[-1] + NCORES * QROWS[q])      # table region starts


def _build_nc(plan):
    """plan: tiles[r][p], call_off[r][p] (tile base), tot_tiles, tmax (per
    pair), tmaxc (per merged call)."""
    tiles = plan["tiles"]
    call_off = plan["call_off"]
    tot_tiles = plan["tot_tiles"]
    TMAX = plan["tmax"]
    TMAXC = plan["tmaxc"]

    nc = bacc.Bacc("TRN2", target_bir_lowering=False, num_devices=NCORES,
                   num_swdge_queues=4)
    f32, b16, i16 = mybir.dt.float32, mybir.dt.bfloat16, mybir.dt.int16
    fp8 = mybir.dt.float8e4

    NBC = NB * 128  # 12544 padded node columns per core

    xT = nc.declare_dram_parameter("xT", [IN, NBC], b16, isOutput=False)
    w1 = nc.declare_dram_parameter("w1", [IN, HID], b16, isOutput=False)
    w2 = nc.declare_dram_parameter("w2", [HID, OUT], b16, isOutput=False)
    w3 = nc.declare_dram_parameter("w3", [HID, OUT], b16, isOutput=False)
    # cols: gamma, beta, b2, b3 (f32 [128, 4])
    crow = nc.declare_dram_parameter("crow", [128, 4], f32, isOutput=False)
    disrep_d = nc.declare_dram_parameter("disrep", [128, NBC], b16,
                                         isOutput=False)
    discol_d = nc.declare_dram_parameter("discol", [128, NB], f32,
                                         isOutput=False)
    dloc_d = nc.declare_dram_parameter("dloct", [128, tot_tiles], b16,
                                       isOutput=False)
    ndloc_d = nc.declare_dram_parameter("ndloct", [128, tot_tiles], b16,
                                        isOutput=False)
    iotar_d = nc.declare_dram_parameter("iotar", [128, TMAXC * 256], b16,
                                        isOutput=False)
    idx_d = nc.declare_dram_parameter("idx16", [128, tot_tiles * 8], i16,
                                      isOutput=False)
    metab = nc.declare_dram_parameter("metab", [128, 129], b16, isOutput=False)
    omu = nc.declare_dram_parameter("omu", [OUT, SH], f32, isOutput=True)
    ols = nc.declare_dram_parameter("ols", [OUT, SH], f32, isOutput=True)

    rg = [list(range(NCORES))]

    with tile.TileContext(nc) as tc:
        with (
            tc.tile_pool(name="const", bufs=1) as cp,
            tc.tile_pool(name="dram", bufs=1, space="DRAM") as dp,
        ):
            w1_t = cp.tile([128, 2, HID], b16)
            w2_t = cp.tile([HID, OUT], b16)
            w3_t = cp.tile([HID, OUT], b16)
            crow_t = cp.tile([128, 4], f32)
            disrep = cp.tile([128, NBC], b16)
            discol = cp.tile([128, NB], f32)
            dloc_t = cp.tile([128, tot_tiles], b16)
            ndloc_t = cp.tile([128, tot_tiles], b16)
            iotar = cp.tile([128, TMAXC * 256], b16)
            metab_t = cp.tile([128, 129], b16)
            z_acc = cp.tile([128, NBC], f32)
            h1p = cp.tile([128, NBC], b16)
            s_parts = cp.tile([128, NPAIR], f32)
            q_parts = cp.tile([128, NPAIR], f32)
            # transform inputs first: the q0 path gates the first AllGather
            nc.sync.dma_start(out=w1_t[:],
                              in_=w1[:].rearrange("(k p) n -> p k n", p=128))
            nc.sync.dma_start(out=discol[:], in_=discol_d[:])
            nc.sync.dma_start(out=metab_t[:], in_=metab[:])
            # bulk constants on the scalar DMA queue (off the critical path)
            nc.scalar.dma_start(out=w2_t[:], in_=w2[:])
            nc.scalar.dma_start(out=w3_t[:], in_=w3[:])
            nc.scalar.dma_start(out=crow_t[:], in_=crow[:])
            nc.scalar.dma_start(out=disrep[:], in_=disrep_d[:])
            nc.scalar.dma_start(out=dloc_t[:], in_=dloc_d[:])
            nc.scalar.dma_start(out=ndloc_t[:], in_=ndloc_d[:])
            nc.scalar.dma_start(out=iotar[:], in_=iotar_d[:])
            ident = cp.tile([128, 128], b16)
            nc.vector.tensor_tensor(
                out=ident[:], in0=metab_t[:, 0:128],
                in1=metab_t[:, 128:129].to_broadcast([128, 128]),
                op=mybir.AluOpType.is_equal)
            gam_c = crow_t[:, 0:1]
            bet_c = crow_t[:, 1:2]
            b2_c = crow_t[0:OUT, 2:3]
            b3_c = crow_t[0:OUT, 3:4]

            sh1_d = dp.tile([SH, HID], b16)
            sh2_d = dp.tile([SH, HID], b16)
            tab1_q = [dp.tile([NCORES * QROWS[q], HID], b16,
                              addr_space="Shared", name=f"tab1q{q}")
                      for q in range(4)]
            tab2_q = [dp.tile([NCORES * QROWS[q], HID], b16,
                              addr_space="Shared", name=f"tab2q{q}")
                      for q in range(4)]
            stats_d = dp.tile([128, 2], f32)
            stats2_d = dp.tile([128, 2], f32)

            # ================= transform + quarter AllGathers ==============
            with (
                tc.tile_pool(name="xq", bufs=2) as xp,
                tc.tile_pool(name="st1", bufs=2) as sp1,
                tc.tile_pool(name="tps", bufs=2, space="PSUM") as tpp,
            ):
                def tf_body(q):
                    nblk = QBLK0[q + 1] - QBLK0[q]
                    xq = xp.tile([128, 2, 3200], b16, tag="xq")
                    nc.sync.dma_start(
                        out=xq[:, :, 0:nblk * 128],
                        in_=xT[:, QBLK0[q] * 128:QBLK0[q + 1] * 128]
                        .rearrange("(k p) n -> p k n", p=128))
                    stage = sp1.tile([128, 3200], b16, tag="st")
                    for bi in range(nblk):
                        b = QBLK0[q] + bi
                        ps = tpp.tile([128, HID], f32, space="PSUM", tag="tp")
                        for kk in range(2):
                            nc.tensor.matmul(
                                out=ps[:],
                                lhsT=xq[:, kk, bi * 128:(bi + 1) * 128],
                                rhs=w1_t[:, kk, :],
                                start=(kk == 0), stop=(kk == 1),
                            )
                        nc.vector.tensor_tensor(
                            out=stage[:, bi * 128:(bi + 1) * 128], in0=ps[:],
                            in1=discol[:, b:b + 1].to_broadcast([128, 128]),
                            op=mybir.AluOpType.mult,
                        )
                    return stage

                for q in range(4):
                    stage = tf_body(q)
                    rows = QROWS[q]
                    full = rows // 128
                    rem = rows - full * 128
                    if full:
                        nc.sync.dma_start(
                            out=sh1_d[QOFF[q]:QOFF[q] + full * 128, :]
                            .rearrange("(b p) f -> p b f", p=128),
                            in_=stage[:, 0:full * 128]
                            .rearrange("p (b f) -> p b f", f=HID),
                        )
                    if rem:
                        nc.sync.dma_start(
                            out=sh1_d[QOFF[q] + full * 128:QOFF[q + 1], :],
                            in_=stage[0:rem, full * 128:full * 128 + HID],
                        )
                    nc.gpsimd.collective_compute(
                        "AllGather", mybir.AluOpType.bypass, replica_groups=rg,
                        ins=[sh1_d[QOFF[q]:QOFF[q + 1], :].opt()],
                        outs=[tab1_q[q][:].opt()],
                    )

            # ===================== aggregation pass ========================
            qctr = [0]

            def agg_pass(tab_q, sh_d, out_cb, pools):
                mp, pp, zp, op, ip = pools
                for r in range(NRANGE):
                    for cg in range(NCG):
                        p0 = cg * GP
                        p1 = min(NPAIR, p0 + GP)
                        cts = [tiles[r][p] for p in range(p0, p1)]
                        ctc = sum(cts)
                        if ctc == 0:
                            continue
                        off = call_off[r][p0]
                        idxt = ip.tile([128, TMAXC * 8], i16, tag="ix")
                        nc.scalar.dma_start(
                            out=idxt[:, 0:ctc * 8],
                            in_=idx_d[:, off * 8:(off + ctc) * 8])
                        mt = mp.tile([128, TMAXC, 128], b16, tag="m")
                        nc.gpsimd.dma_gather(
                            out_ap=mt[:, 0:ctc, :],
                            in_ap=tab_q[r][:],
                            idxs_ap=idxt[:, 0:ctc * 8],
                            num_idxs=ctc * 128,
                            num_idxs_reg=ctc * 128,
                            elem_size=HID,
                            single_packet=False,
                            queue_num=qctr[0] % 4,
                        )
                        qctr[0] += 1
                        pt = pp.tile([128, TMAXC, 256], fp8, tag="p")
                        nv = max(0, ctc - SCT)
                        if nv:
                            nc.vector.tensor_tensor(
                                out=pt[:, 0:nv, :],
                                in0=iotar[:, 0:nv * 256]
                                .rearrange("p (t f) -> p t f", f=256),
                                in1=dloc_t[:, off:off + nv]
                                .to_broadcast([128, nv, 256]),
                                op=mybir.AluOpType.is_equal,
                            )
                        for t in range(nv, ctc):
                            sq = op.tile([128, 256], b16, tag="sq")
                            nc.scalar.activation(
                                out=sq[:], in_=iotar[:, 0:256],
                                func=mybir.ActivationFunctionType.Square,
                                bias=ndloc_t[:, off + t:off + t + 1])
                            nc.scalar.activation(
                                out=pt[:, t, :], in_=sq[:],
                                func=mybir.ActivationFunctionType.Relu,
                                bias=1.0, scale=-1.0)
                        tb = 0
                        for pi, p in enumerate(range(p0, p1)):
                            ct = cts[pi]
                            if ct == 0:
                                continue
                            zc = zp.tile([128, 256], f32, space="PSUM",
                                         tag="z", name="zc")
                            if r == 0:
                                owns = []
                                for j in range(2):
                                    b = p * 2 + j
                                    lo = b * 128
                                    hi = min(SH, lo + 128)
                                    own = op.tile([128, HID], b16,
                                                  tag=f"own{j}")
                                    nc.scalar.dma_start(
                                        out=own[0:hi - lo, :],
                                        in_=sh_d[lo:hi, :])
                                    owns.append((own, hi - lo))
                            for t in range(ct):
                                if r == 0 and t == ct - 1:
                                    for j in range(2):
                                        own, nrow = owns[j]
                                        nc.tensor.matmul(
                                            out=zc[:, j * 128:(j + 1) * 128],
                                            lhsT=own[0:nrow, :],
                                            rhs=ident[0:nrow, :],
                                            start=False, stop=False,
                                        )
                                nc.tensor.matmul(
                                    out=zc[:],
                                    lhsT=mt[:, tb + t, :],
                                    rhs=pt[:, tb + t, :],
                                    start=(t == 0),
                                    stop=(t == ct - 1),
                                )
                            tb += ct
                            pc = p * 256
                            if r == 0:
                                nc.vector.tensor_copy(
                                    out=z_acc[:, pc:pc + 256], in_=zc[:])
                            else:
                                nc.vector.tensor_tensor(
                                    out=z_acc[:, pc:pc + 256],
                                    in0=z_acc[:, pc:pc + 256], in1=zc[:],
                                    op=mybir.AluOpType.add)
                                if r == NRANGE - 1:
                                    out_cb(p)

            # ---- pass 1 ----
            with (
                tc.tile_pool(name="mb", bufs=3) as mp1,
                tc.tile_pool(name="pb", bufs=3) as pp1,
                tc.tile_pool(name="zps", bufs=4, space="PSUM") as zp1,
                tc.tile_pool(name="ow", bufs=4) as op1,
                tc.tile_pool(name="ix", bufs=3) as ip1,
                tc.tile_pool(name="sqp", bufs=2) as sqp,
            ):
                def out1(p):
                    pc = p * 256
                    nc.vector.tensor_tensor(
                        out=h1p[:, pc:pc + 256], in0=z_acc[:, pc:pc + 256],
                        in1=disrep[:, pc:pc + 256],
                        op=mybir.AluOpType.mult,
                    )
                    nc.vector.tensor_reduce(
                        out=s_parts[:, p:p + 1], in_=h1p[:, pc:pc + 256],
                        axis=mybir.AxisListType.X, op=mybir.AluOpType.add)
                    sq = sqp.tile([128, 256], f32, tag="sq", name="sqe")
                    nc.scalar.square(out=sq[:], in_=h1p[:, pc:pc + 256])
                    nc.vector.tensor_reduce(
                        out=q_parts[:, p:p + 1], in_=sq[:],
                        axis=mybir.AxisListType.X, op=mybir.AluOpType.add)

                agg_pass(tab1_q, sh1_d, out1, (mp1, pp1, zp1, op1, ip1))

                # ---- BN stats + AllReduce ----
                with tc.tile_pool(name="bn", bufs=1) as bp:
                    s_col = bp.tile([128, 1], f32)
                    q_col = bp.tile([128, 1], f32)
                    nc.vector.tensor_reduce(
                        out=s_col[:], in_=s_parts[:], axis=mybir.AxisListType.X,
                        op=mybir.AluOpType.add)
                    nc.vector.tensor_reduce(
                        out=q_col[:], in_=q_parts[:], axis=mybir.AxisListType.X,
                        op=mybir.AluOpType.add)
                    nc.sync.dma_start(out=stats_d[:, 0:1], in_=s_col[:])
                    nc.sync.dma_start(out=stats_d[:, 1:2], in_=q_col[:])
                    nc.gpsimd.collective_compute(
                        "AllReduce", mybir.AluOpType.add, replica_groups=rg,
                        ins=[stats_d[:].opt()], outs=[stats2_d[:].opt()],
                    )
                    st = bp.tile([128, 2], f32)
                    nc.sync.dma_start(out=st[:], in_=stats2_d[:])
                    mean = bp.tile([128, 1], f32)
                    ex2 = bp.tile([128, 1], f32)
                    msq = bp.tile([128, 1], f32)
                    var = bp.tile([128, 1], f32)
                    std = bp.tile([128, 1], f32)
                    inv = bp.tile([128, 1], f32)
                    a_col = bp.tile([128, 1], f32)
                    bm = bp.tile([128, 1], f32)
                    b_col = bp.tile([128, 1], f32)
                    nc.vector.tensor_scalar(
                        out=mean[:], in0=st[:, 0:1], scalar1=1.0 / N,
                        scalar2=None, op0=mybir.AluOpType.mult)
                    nc.vector.tensor_scalar(
                        out=ex2[:], in0=st[:, 1:2], scalar1=1.0 / N,
                        scalar2=None, op0=mybir.AluOpType.mult)
                    nc.vector.tensor_tensor(
                        out=msq[:], in0=mean[:], in1=mean[:],
                        op=mybir.AluOpType.mult)
                    nc.vector.tensor_tensor(
                        out=var[:], in0=ex2[:], in1=msq[:],
                        op=mybir.AluOpType.subtract)
                    nc.vector.tensor_scalar(
                        out=var[:], in0=var[:], scalar1=BN_EPS, scalar2=None,
                        op0=mybir.AluOpType.add)
                    nc.scalar.activation(
                        out=std[:], in_=var[:],
                        func=mybir.ActivationFunctionType.Sqrt, bias=0.0)
                    nc.vector.reciprocal(out=inv[:], in_=std[:])
                    nc.vector.tensor_tensor(
                        out=a_col[:], in0=gam_c, in1=inv[:],
                        op=mybir.AluOpType.mult)
                    nc.vector.tensor_tensor(
                        out=bm[:], in0=mean[:], in1=a_col[:],
                        op=mybir.AluOpType.mult)
                    nc.vector.tensor_tensor(
                        out=b_col[:], in0=bet_c, in1=bm[:],
                        op=mybir.AluOpType.subtract)

                    # ---- table2 = relu(BN(h1p)) * dis, transposed out ----
                    with (
                        tc.tile_pool(name="t2", bufs=3) as t2p,
                        tc.tile_pool(name="st2", bufs=2) as sp2,
                        tc.tile_pool(name="t2ps", bufs=2, space="PSUM") as t2pp,
                    ):
                        for q in range(4):
                            nblk = QBLK0[q + 1] - QBLK0[q]
                            stage = sp2.tile([128, 3200], b16, tag="st")
                            for bi in range(0, nblk, 2):
                                b = QBLK0[q] + bi
                                bc = b * 128
                                w2b = min(nblk - bi, 2) * 128
                                u = t2p.tile([128, 256], b16, tag="u")
                                nc.scalar.activation(
                                    out=u[:, 0:w2b], in_=h1p[:, bc:bc + w2b],
                                    func=mybir.ActivationFunctionType.Relu,
                                    bias=b_col[:], scale=a_col[:])
                                u2 = t2p.tile([128, 256], b16, tag="u2")
                                nc.vector.tensor_tensor(
                                    out=u2[:, 0:w2b], in0=u[:, 0:w2b],
                                    in1=disrep[:, bc:bc + w2b],
                                    op=mybir.AluOpType.mult)
                                for jj in range(w2b // 128):
                                    psT = t2pp.tile([128, 128], b16,
                                                    space="PSUM", tag="pt")
                                    nc.tensor.transpose(
                                        out=psT[:],
                                        in_=u2[:, jj * 128:(jj + 1) * 128],
                                        identity=ident[:])
                                    nc.scalar.activation(
                                        out=stage[:, (bi + jj) * 128:
                                                  (bi + jj + 1) * 128],
                                        in_=psT[:],
                                        func=mybir.ActivationFunctionType.Copy,
                                        bias=0.0)
                            rows = QROWS[q]
                            full = rows // 128
                            rem = rows - full * 128
                            if full:
                                nc.sync.dma_start(
                                    out=sh2_d[QOFF[q]:QOFF[q] + full * 128, :]
                                    .rearrange("(b p) f -> p b f", p=128),
                                    in_=stage[:, 0:full * 128]
                                    .rearrange("p (b f) -> p b f", f=HID),
                                )
                            if rem:
                                nc.sync.dma_start(
                                    out=sh2_d[QOFF[q] + full * 128:QOFF[q + 1], :],
                                    in_=stage[0:rem,
                                              full * 128:full * 128 + HID],
                                )
                            nc.gpsimd.collective_compute(
                                "AllGather", mybir.AluOpType.bypass,
                                replica_groups=rg,
                                ins=[sh2_d[QOFF[q]:QOFF[q + 1], :].opt()],
                                outs=[tab2_q[q][:].opt()],
                            )

                # ---- pass 2 + heads ----
                with (
                    tc.tile_pool(name="hd", bufs=2) as hp,
                    tc.tile_pool(name="hst", bufs=2) as hsp,
                    tc.tile_pool(name="hps", bufs=2, space="PSUM") as hpp,
                ):
                    def out2(p):
                        pc = p * 256
                        w = min(SH, pc + 256) - pc
                        zt = hp.tile([128, 256], b16, tag="zt")
                        nc.vector.tensor_tensor(
                            out=zt[:], in0=z_acc[:, pc:pc + 256],
                            in1=disrep[:, pc:pc + 256],
                            op=mybir.AluOpType.mult)
                        hps = hpp.tile([OUT, 512], f32, space="PSUM",
                                       tag="hps", name="hps")
                        nc.tensor.matmul(out=hps[:, 0:256], lhsT=w2_t[:],
                                         rhs=zt[:], start=True, stop=True)
                        nc.tensor.matmul(out=hps[:, 256:512], lhsT=w3_t[:],
                                         rhs=zt[:], start=True, stop=True)
                        smu = hsp.tile([OUT, 256], f32, tag="smu")
                        sls = hsp.tile([OUT, 256], f32, tag="sls")
                        nc.scalar.activation(
                            out=smu[:], in_=hps[:, 0:256],
                            func=mybir.ActivationFunctionType.Identity,
                            bias=b2_c)
                        nc.scalar.activation(
                            out=sls[:], in_=hps[:, 256:512],
                            func=mybir.ActivationFunctionType.Identity,
                            bias=b3_c)
                        nc.sync.dma_start(out=omu[:, pc:pc + w],
                                          in_=smu[:, 0:w])
                        nc.sync.dma_start(out=ols[:, pc:pc + w],
                                          in_=sls[:, 0:w])

                    agg_pass(tab2_q, sh2_d, out2, (mp1, pp1, zp1, op1, ip1))

    nc.compile()
    return nc


def _preprocess(x, edge_index, W1, b1, gamma, beta, W2, b2, W3, b3):
    src = np.asarray(edge_index[0], dtype=np.int64)
    dst = np.asarray(edge_index[1], dtype=np.int64)
    deg = 1.0 + np.bincount(dst, minlength=N).astype(np.float64)
    dis = (1.0 / np.sqrt(deg)).astype(np.float32)

    core = dst // SH
    dl = dst % SH
    pair = dl // 256
    dloc = dl % 256

    sc = src // SH
    sloc = src % SH
    q = np.digitize(sloc, QOFF[1:4])            # 0..3
    qrows = np.asarray(QROWS)[q]
    rel = (sc * qrows + sloc - np.asarray(QOFF[:4])[q]).astype(np.int64)

    # group key: (core, range, pair)  -- range-major slot space
    gkey = (core * NRANGE + q) * NPAIR + pair
    ngroups = NCORES * NRANGE * NPAIR
    cnt = np.bincount(gkey, minlength=ngroups)
    order = np.argsort(gkey, kind="stable")
    start = np.zeros(ngroups + 1, np.int64)
    np.cumsum(cnt, out=start[1:])
    rank = np.arange(len(gkey)) - start[gkey[order]]
    cntm = cnt.reshape(NCORES, NRANGE, NPAIR)
    gt = ((cntm.max(axis=0) + 127) // 128)      # tiles[range][pair]
    tot_tiles = int(gt.sum())
    call_off = np.zeros((NRANGE, NPAIR), np.int64)
    flat = gt.reshape(-1)
    call_off.reshape(-1)[1:] = np.cumsum(flat)[:-1]
    tmax = int(gt.max())
    tmaxc = 0
    for rr in range(NRANGE):
        for cg in range(NCG):
            tmaxc = max(tmaxc, int(gt[rr][cg * GP:(cg + 1) * GP].sum()))

    # slot for each edge: group's tile base * 128 + rank
    gkey_o = gkey[order]
    r_o = (gkey_o // NPAIR) % NRANGE
    p_o = gkey_o % NPAIR
    slot = call_off[r_o, p_o] * 128 + rank
    core_o = gkey_o // (NRANGE * NPAIR)

    plan = dict(
        tiles=[[int(gt[rr][pp]) for pp in range(NPAIR)]
               for rr in range(NRANGE)],
        call_off=[[int(call_off[rr][pp]) for pp in range(NPAIR)]
                  for rr in range(NRANGE)],
        tot_tiles=tot_tiles,
        tmax=tmax,
        tmaxc=tmaxc,
    )

    rel_o = rel[order]
    dloc_o = dloc[order]

    in_maps = []
    tot_slots = tot_tiles * 128
    iota = np.tile(np.arange(128, dtype=np.float32), (128, 1))
    pcol = np.arange(128, dtype=np.float32).reshape(128, 1)
    metab_a = np.concatenate([iota, pcol], axis=1).astype(bf16)
    iotar_a = np.tile(np.arange(256, dtype=np.float32),
                      (128, tmaxc)).astype(bf16)
    for c in range(NCORES):
        m = core_o == c
        idx_flat = np.zeros(tot_slots, np.int16)
        idx_flat[slot[m]] = rel_o[m].astype(np.int16)

        arr = idx_flat.reshape(tot_slots // 16, 16).T.copy()  # [16, S/16]
        idx16_a = np.tile(arr, (8, 1))

        dloc_a = np.full((128, tot_tiles), 512.0, np.float32)
        sl = slot[m]
        dloc_a[sl % 128, sl // 128] = dloc_o[m]

        base = c * SH
        disn = np.zeros(NB * 128, np.float32)
        disn[:SH] = dis[base:base + SH]
        disrep_a = np.tile(disn[None, :], (128, 1)).astype(bf16)
        discol_a = disn.reshape(NB, 128).T.copy()

        crow_a = np.zeros((128, 4), np.float32)
        crow_a[:, 0] = np.asarray(gamma, np.float32)
        crow_a[:, 1] = np.asarray(beta, np.float32)
        crow_a[:OUT, 2] = np.asarray(b2, np.float32)
        crow_a[:OUT, 3] = np.asarray(b3, np.float32)

        xs = np.asarray(x[base:base + SH], np.float32)
        xT_a = np.zeros((IN, NB * 128), np.float32)
        xT_a[:, :SH] = xs.T
        in_maps.append(dict(
            xT=xT_a.astype(bf16),
            w1=np.asarray(W1, np.float32).astype(bf16),
            w2=np.asarray(W2, np.float32).astype(bf16),
            w3=np.asarray(W3, np.float32).astype(bf16),
            crow=crow_a,
            disrep=disrep_a,
            discol=discol_a,
            dloct=dloc_a.astype(bf16),
            ndloct=(-dloc_a).astype(bf16),
            iotar=iotar_a,
            idx16=idx16_a,
            metab=metab_a,
        ))
    return in_maps, plan


_NC_CACHE = {}


def kernel(**inputs):
    in_maps, plan = _preprocess(**inputs)
    key = (plan["tot_tiles"], plan["tmax"], plan["tmaxc"])
    if key not in _NC_CACHE:
        _NC_CACHE[key] = _build_nc(plan)
    nc = _NC_CACHE[key]
    res = run_bass_kernel_spmd(nc, in_maps, core_ids=list(range(NCORES)))
    xm = np.concatenate([res.results[c]["omu"].T for c in range(NCORES)], axis=0)
    x_ = np.concatenate([res.results[c]["ols"].T for c in range(NCORES)], axis=0)
    return xm.astype(np.float32), x_.astype(np.float32)


# revision 15
# speedup vs baseline: 1.3517x; 1.3517x over previous
"""GCN encoder (conv->BN->ReLU->2 conv heads) on 8 TRN2 NeuronCores.

Sharding: nodes (dst) split 8 ways. Each layer computes
agg[d] = dis_d * (sum_{e: dst=d} hs[src_e] + hs[d]) with hs = (h@W)*dis.
Per layer each core transforms its shard, publishes bf16 table rows via 4
quarter-wise AllGathers (overlapped with transform), then aggregates its dst
shard RANGE-MAJOR into an SBUF f32 accumulator: for each source quarter r,
merged gather calls (4 dst pairs each, int16 indices, 4 SWDGE queues round-
robin) pull per-edge source rows; one-hot P tiles (fp8, 256 dst cols) are
built on-chip from a compact per-slot dst-index table (DVE is_equal for most
tiles, scalar-engine relu(1-(x-d)^2) for the tail to balance engines); PE
accumulates z[feat, dst] += M_t^T @ P_t in PSUM per (pair, r); DVE folds each
round into z_acc. Self-loop terms enter via identity matmuls on the core's
own staged rows during round 0. Range-major order lets round r start as soon
as AllGather(r) lands, hiding the collective latency at both layer starts.
Evictions fuse the dis scale and BN stats; BN+ReLU for table2 is a scalar
activation chain around a PE transpose; head matmuls consume z [feat, dst]
directly with biases folded into the PSUM->SBUF activation copy.
"""

import os
import sys

sys.path.insert(0, "/opt/trn_rl_repo")

import numpy as np
import ml_dtypes

from concourse import bacc, bass, mybir, tile
from concourse.bass_utils import run_bass_kernel_spmd

bf16 = ml_dtypes.bfloat16

N = 100000
E = 1600000
IN = 256
HID = 128
OUT = 64
BN_EPS = 1e-5
NCORES = 8
SH = N // NCORES            # 12500 nodes per core
NB = (SH + 127) // 128      # 98 dst blocks (last has 84 nodes)
NPAIR = NB // 2             # 49 dst pairs (256 cols each)
NRANGE = 4
GP = 1                      # pairs per gather call
NCG = (NPAIR + GP - 1) // GP    # call groups
SCT = 2                     # tail tiles per call generated on scalar engine
# quarter split of each core's shard (block-aligned starts; q3 short)
QOFF = [0, 3200, 6400, 9600, SH]
QROWS = [QOFF[i + 1] - QOFF[i] for i in range(4)]      # 3200,3200,3200,2900
QBLK0 = [0, 25, 50, 75, NB]                            # block ranges per quarter
RSTART = [0]
for q in range(4):
    RSTART.append(RSTART[-1] + NCORES * QROWS[q])      # table region starts


def _build_nc(plan):
    """plan: tiles[r][p], call_off[r][p] (tile base), tot_tiles, tmax (per
    pair), tmaxc (per merged call)."""
    tiles = plan["tiles"]
    call_off = plan["call_off"]
    tot_tiles = plan["tot_tiles"]
    TMAX = plan["tmax"]
    TMAXC = plan["tmaxc"]

    nc = bacc.Bacc("TRN2", target_bir_lowering=False, num_devices=NCORES,
                   num_swdge_queues=4)
    f32, b16, i16 = mybir.dt.float32, mybir.dt.bfloat16, mybir.dt.int16
    fp8 = mybir.dt.float8e4

    NBC = NB * 128  # 12544 padded node columns per core

    xT = nc.declare_dram_parameter("xT", [IN, NBC], b16, isOutput=False)
    w1 = nc.declare_dram_parameter("w1", [IN, HID], b16, isOutput=False)
    w2 = nc.declare_dram_parameter("w2", [HID, OUT], b16, isOutput=False)
    w3 = nc.declare_dram_parameter("w3", [HID, OUT], b16, isOutput=False)
    # cols: gamma, beta, b2, b3 (f32 [128, 4])
    crow = nc.declare_dram_parameter("crow", [128, 4], f32, isOutput=False)
    disrep_d = nc.declare_dram_parameter("disrep", [128, NBC], b16,
                                         isOutput=False)
    discol_d = nc.declare_dram_parameter("discol", [128, NB], f32,
                                         isOutput=False)
    dloc_d = nc.declare_dram_parameter("dloct", [128, tot_tiles], b16,
                                       isOutput=False)
    ndloc_d = nc.declare_dram_parameter("ndloct", [128, tot_tiles], b16,
                                        isOutput=False)
    iotar_d = nc.declare_dram_parameter("iotar", [128, TMAXC * 256], b16,
                                        isOutput=False)
    idx_d = nc.declare_dram_parameter("idx16", [128, tot_tiles * 8], i16,
                                      isOutput=False)
    metab = nc.declare_dram_parameter("metab", [128, 129], b16, isOutput=False)
    omu = nc.declare_dram_parameter("omu", [OUT, SH], f32, isOutput=True)
    ols = nc.declare_dram_parameter("ols", [OUT, SH], f32, isOutput=True)

    rg = [list(range(NCORES))]

    with tile.TileContext(nc) as tc:
        with (
            tc.tile_pool(name="const", bufs=1) as cp,
            tc.tile_pool(name="dram", bufs=1, space="DRAM") as dp,
        ):
            w1_t = cp.tile([128, 2, HID], b16)
            w2_t = cp.tile([HID, OUT], b16)
            w3_t = cp.tile([HID, OUT], b16)
            crow_t = cp.tile([128, 4], f32)
            disrep = cp.tile([128, NBC], b16)
            discol = cp.tile([128, NB], f32)
            dloc_t = cp.tile([128, tot_tiles], b16)
            ndloc_t = cp.tile([128, tot_tiles], b16)
            iotar = cp.tile([128, TMAXC * 256], b16)
            metab_t = cp.tile([128, 129], b16)
            z_acc = cp.tile([128, NBC], f32)
            h1p = cp.tile([128, NBC], b16)
            s_parts = cp.tile([128, NPAIR], f32)
            q_parts = cp.tile([128, NPAIR], f32)
            # transform inputs first: the q0 path gates the first AllGather
            nc.sync.dma_start(out=w1_t[:],
                              in_=w1[:].rearrange("(k p) n -> p k n", p=128))
            nc.sync.dma_start(out=discol[:], in_=discol_d[:])
            nc.sync.dma_start(out=metab_t[:], in_=metab[:])
            # bulk constants on the scalar DMA queue (off the critical path)
            nc.scalar.dma_start(out=w2_t[:], in_=w2[:])
            nc.scalar.dma_start(out=w3_t[:], in_=w3[:])
            nc.scalar.dma_start(out=crow_t[:], in_=crow[:])
            nc.scalar.dma_start(out=disrep[:], in_=disrep_d[:])
            nc.scalar.dma_start(out=dloc_t[:], in_=dloc_d[:])
            nc.scalar.dma_start(out=ndloc_t[:], in_=ndloc_d[:])
            nc.scalar.dma_start(out=iotar[:], in_=iotar_d[:])
            ident = cp.tile([128, 128], b16)
            nc.vector.tensor_tensor(
                out=ident[:], in0=metab_t[:, 0:128],
                in1=metab_t[:, 128:129].to_broadcast([128, 128]),
                op=mybir.AluOpType.is_equal)
            gam_c = crow_t[:, 0:1]
            bet_c = crow_t[:, 1:2]
            b2_c = crow_t[0:OUT, 2:3]
            b3_c = crow_t[0:OUT, 3:4]

            sh1_d = dp.tile([SH, HID], b16)
            sh2_d = dp.tile([SH, HID], b16)
            tab1_q = [dp.tile([NCORES * QROWS[q], HID], b16,
                              addr_space="Shared", name=f"tab1q{q}")
                      for q in range(4)]
            tab2_q = [dp.tile([NCORES * QROWS[q], HID], b16,
                              addr_space="Shared", name=f"tab2q{q}")
                      for q in range(4)]
            stats_d = dp.tile([128, 2], f32)
            stats2_d = dp.tile([128, 2], f32)

            # ================= transform + quarter AllGathers ==============
            with (
                tc.tile_pool(name="xq", bufs=2) as xp,
                tc.tile_pool(name="st1", bufs=2) as sp1,
                tc.tile_pool(name="tps", bufs=2, space="PSUM") as tpp,
            ):
                def tf_body(q):
                    nblk = QBLK0[q + 1] - QBLK0[q]
                    xq = xp.tile([128, 2, 3200], b16, tag="xq")
                    nc.sync.dma_start(
                        out=xq[:, :, 0:nblk * 128],
                        in_=xT[:, QBLK0[q] * 128:QBLK0[q + 1] * 128]
                        .rearrange("(k p) n -> p k n", p=128))
                    stage = sp1.tile([128, 3200], b16, tag="st")
                    for bi in range(nblk):
                        b = QBLK0[q] + bi
                        ps = tpp.tile([128, HID], f32, space="PSUM", tag="tp")
                        for kk in range(2):
                            nc.tensor.matmul(
                                out=ps[:],
                                lhsT=xq[:, kk, bi * 128:(bi + 1) * 128],
                                rhs=w1_t[:, kk, :],
                                start=(kk == 0), stop=(kk == 1),
                            )
                        nc.vector.tensor_tensor(
                            out=stage[:, bi * 128:(bi + 1) * 128], in0=ps[:],
                            in1=discol[:, b:b + 1].to_broadcast([128, 128]),
                            op=mybir.AluOpType.mult,
                        )
                    return stage

                for q in range(4):
                    stage = tf_body(q)
                    rows = QROWS[q]
                    full = rows // 128
                    rem = rows - full * 128
                    if full:
                        nc.sync.dma_start(
                            out=sh1_d[QOFF[q]:QOFF[q] + full * 128, :]
                            .rearrange("(b p) f -> p b f", p=128),
                            in_=stage[:, 0:full * 128]
                            .rearrange("p (b f) -> p b f", f=HID),
                        )
                    if rem:
                        nc.sync.dma_start(
                            out=sh1_d[QOFF[q] + full * 128:QOFF[q + 1], :],
                            in_=stage[0:rem, full * 128:full * 128 + HID],
                        )
                    nc.gpsimd.collective_compute(
                        "AllGather", mybir.AluOpType.bypass, replica_groups=rg,
                        ins=[sh1_d[QOFF[q]:QOFF[q + 1], :].opt()],
                        outs=[tab1_q[q][:].opt()],
                    )

            # ===================== aggregation pass ========================
            qctr = [0]

            def agg_pass(tab_q, sh_d, out_cb, pools):
                mp, pp, zp, op, ip = pools
                for r in range(NRANGE):
                    for cg in range(NCG):
                        p0 = cg * GP
                        p1 = min(NPAIR, p0 + GP)
                        cts = [tiles[r][p] for p in range(p0, p1)]
                        ctc = sum(cts)
                        if ctc == 0:
                            continue
                        off = call_off[r][p0]
                        lane = cg % 4
                        idxt = ip.tile([128, TMAXC * 8], i16, tag=f"ix{lane}")
                        nc.scalar.dma_start(
                            out=idxt[:, 0:ctc * 8],
                            in_=idx_d[:, off * 8:(off + ctc) * 8])
                        mt = mp.tile([128, TMAXC, 128], b16, tag=f"m{lane}")
                        nc.gpsimd.dma_gather(
                            out_ap=mt[:, 0:ctc, :],
                            in_ap=tab_q[r][:],
                            idxs_ap=idxt[:, 0:ctc * 8],
                            num_idxs=ctc * 128,
                            num_idxs_reg=ctc * 128,
                            elem_size=HID,
                            single_packet=False,
                            queue_num=lane,
                        )
                        qctr[0] += 1
                        pt = pp.tile([128, TMAXC, 256], fp8, tag=f"p{lane}")
                        nv = max(0, ctc - SCT)
                        if nv:
                            nc.vector.tensor_tensor(
                                out=pt[:, 0:nv, :],
                                in0=iotar[:, 0:nv * 256]
                                .rearrange("p (t f) -> p t f", f=256),
                                in1=dloc_t[:, off:off + nv]
                                .to_broadcast([128, nv, 256]),
                                op=mybir.AluOpType.is_equal,
                            )
                        for t in range(nv, ctc):
                            sq = op.tile([128, 256], b16, tag="sq")
                            nc.scalar.activation(
                                out=sq[:], in_=iotar[:, 0:256],
                                func=mybir.ActivationFunctionType.Square,
                                bias=ndloc_t[:, off + t:off + t + 1])
                            nc.scalar.activation(
                                out=pt[:, t, :], in_=sq[:],
                                func=mybir.ActivationFunctionType.Relu,
                                bias=1.0, scale=-1.0)
                        tb = 0
                        for pi, p in enumerate(range(p0, p1)):
                            ct = cts[pi]
                            if ct == 0:
                                continue
                            zc = zp.tile([128, 256], f32, space="PSUM",
                                         tag="z", name="zc")
                            if r == 0:
                                owns = []
                                for j in range(2):
                                    b = p * 2 + j
                                    lo = b * 128
                                    hi = min(SH, lo + 128)
                                    own = op.tile([128, HID], b16,
                                                  tag=f"own{j}")
                                    nc.scalar.dma_start(
                                        out=own[0:hi - lo, :],
                                        in_=sh_d[lo:hi, :])
                                    owns.append((own, hi - lo))
                            for t in range(ct):
                                if r == 0 and t == ct - 1:
                                    for j in range(2):
                                        own, nrow = owns[j]
                                        nc.tensor.matmul(
                                            out=zc[:, j * 128:(j + 1) * 128],
                                            lhsT=own[0:nrow, :],
                                            rhs=ident[0:nrow, :],
                                            start=False, stop=False,
                                        )
                                nc.tensor.matmul(
                                    out=zc[:],
                                    lhsT=mt[:, tb + t, :],
                                    rhs=pt[:, tb + t, :],
                                    start=(t == 0),
                                    stop=(t == ct - 1),
                                )
                            tb += ct
                            pc = p * 256
                            if r == 0:
                                nc.vector.tensor_copy(
                                    out=z_acc[:, pc:pc + 256], in_=zc[:])
                            else:
                                nc.vector.tensor_tensor(
                                    out=z_acc[:, pc:pc + 256],
                                    in0=z_acc[:, pc:pc + 256], in1=zc[:],
                                    op=mybir.AluOpType.add)
                                if r == NRANGE - 1:
                                    out_cb(p)

            # ---- pass 1 ----
            with (
                tc.tile_pool(name="mb", bufs=3) as mp1,
                tc.tile_pool(name="pb", bufs=3) as pp1,
                tc.tile_pool(name="zps", bufs=4, space="PSUM") as zp1,
                tc.tile_pool(name="ow", bufs=4) as op1,
                tc.tile_pool(name="ix", bufs=3) as ip1,
                tc.tile_pool(name="sqp", bufs=2) as sqp,
            ):
                def out1(p):
                    pc = p * 256
                    nc.vector.tensor_tensor(
                        out=h1p[:, pc:pc + 256], in0=z_acc[:, pc:pc + 256],
                        in1=disrep[:, pc:pc + 256],
                        op=mybir.AluOpType.mult,
                    )
                    nc.vector.tensor_reduce(
                        out=s_parts[:, p:p + 1], in_=h1p[:, pc:pc + 256],
                        axis=mybir.AxisListType.X, op=mybir.AluOpType.add)
                    sq = sqp.tile([128, 256], f32, tag="sq", name="sqe")
                    nc.scalar.square(out=sq[:], in_=h1p[:, pc:pc + 256])
                    nc.vector.tensor_reduce(
                        out=q_parts[:, p:p + 1], in_=sq[:],
                        axis=mybir.AxisListType.X, op=mybir.AluOpType.add)

                agg_pass(tab1_q, sh1_d, out1, (mp1, pp1, zp1, op1, ip1))

                # ---- BN stats + AllReduce ----
                with tc.tile_pool(name="bn", bufs=1) as bp:
                    s_col = bp.tile([128, 1], f32)
                    q_col = bp.tile([128, 1], f32)
                    nc.vector.tensor_reduce(
                        out=s_col[:], in_=s_parts[:], axis=mybir.AxisListType.X,
                        op=mybir.AluOpType.add)
                    nc.vector.tensor_reduce(
                        out=q_col[:], in_=q_parts[:], axis=mybir.AxisListType.X,
                        op=mybir.AluOpType.add)
                    nc.sync.dma_start(out=stats_d[:, 0:1], in_=s_col[:])
                    nc.sync.dma_start(out=stats_d[:, 1:2], in_=q_col[:])
                    nc.gpsimd.collective_compute(
                        "AllReduce", mybir.AluOpType.add, replica_groups=rg,
                        ins=[stats_d[:].opt()], outs=[stats2_d[:].opt()],
                    )
                    st = bp.tile([128, 2], f32)
                    nc.sync.dma_start(out=st[:], in_=stats2_d[:])
                    mean = bp.tile([128, 1], f32)
                    ex2 = bp.tile([128, 1], f32)
                    msq = bp.tile([128, 1], f32)
                    var = bp.tile([128, 1], f32)
                    std = bp.tile([128, 1], f32)
                    inv = bp.tile([128, 1], f32)
                    a_col = bp.tile([128, 1], f32)
                    bm = bp.tile([128, 1], f32)
                    b_col = bp.tile([128, 1], f32)
                    nc.vector.tensor_scalar(
                        out=mean[:], in0=st[:, 0:1], scalar1=1.0 / N,
                        scalar2=None, op0=mybir.AluOpType.mult)
                    nc.vector.tensor_scalar(
                        out=ex2[:], in0=st[:, 1:2], scalar1=1.0 / N,
                        scalar2=None, op0=mybir.AluOpType.mult)
                    nc.vector.tensor_tensor(
                        out=msq[:], in0=mean[:], in1=mean[:],
                        op=mybir.AluOpType.mult)
                    nc.vector.tensor_tensor(
                        out=var[:], in0=ex2[:], in1=msq[:],
                        op=mybir.AluOpType.subtract)
                    nc.vector.tensor_scalar(
                        out=var[:], in0=var[:], scalar1=BN_EPS, scalar2=None,
                        op0=mybir.AluOpType.add)
                    nc.scalar.activation(
                        out=std[:], in_=var[:],
                        func=mybir.ActivationFunctionType.Sqrt, bias=0.0)
                    nc.vector.reciprocal(out=inv[:], in_=std[:])
                    nc.vector.tensor_tensor(
                        out=a_col[:], in0=gam_c, in1=inv[:],
                        op=mybir.AluOpType.mult)
                    nc.vector.tensor_tensor(
                        out=bm[:], in0=mean[:], in1=a_col[:],
                        op=mybir.AluOpType.mult)
                    nc.vector.tensor_tensor(
                        out=b_col[:], in0=bet_c, in1=bm[:],
                        op=mybir.AluOpType.subtract)

                    # ---- table2 = relu(BN(h1p)) * dis, transposed out ----
                    with (
                        tc.tile_pool(name="t2", bufs=3) as t2p,
                        tc.tile_pool(name="st2", bufs=2) as sp2,
                        tc.tile_pool(name="t2ps", bufs=2, space="PSUM") as t2pp,
                    ):
                        for q in range(4):
                            nblk = QBLK0[q + 1] - QBLK0[q]
                            stage = sp2.tile([128, 3200], b16, tag="st")
                            for bi in range(0, nblk, 2):
                                b = QBLK0[q] + bi
                                bc = b * 128
                                w2b = min(nblk - bi, 2) * 128
                                u = t2p.tile([128, 256], b16, tag="u")
                                nc.scalar.activation(
                                    out=u[:, 0:w2b], in_=h1p[:, bc:bc + w2b],
                                    func=mybir.ActivationFunctionType.Relu,
                                    bias=b_col[:], scale=a_col[:])
                                u2 = t2p.tile([128, 256], b16, tag="u2")
                                nc.vector.tensor_tensor(
                                    out=u2[:, 0:w2b], in0=u[:, 0:w2b],
                                    in1=disrep[:, bc:bc + w2b],
                                    op=mybir.AluOpType.mult)
                                for jj in range(w2b // 128):
                                    psT = t2pp.tile([128, 128], b16,
                                                    space="PSUM", tag="pt")
                                    nc.tensor.transpose(
                                        out=psT[:],
                                        in_=u2[:, jj * 128:(jj + 1) * 128],
                                        identity=ident[:])
                                    nc.scalar.activation(
                                        out=stage[:, (bi + jj) * 128:
                                                  (bi + jj + 1) * 128],
                                        in_=psT[:],
                                        func=mybir.ActivationFunctionType.Copy,
                                        bias=0.0)
                            rows = QROWS[q]
                            full = rows // 128
                            rem = rows - full * 128
                            if full:
                                nc.sync.dma_start(
                                    out=sh2_d[QOFF[q]:QOFF[q] + full * 128, :]
                                    .rearrange("(b p) f -> p b f", p=128),
                                    in_=stage[:, 0:full * 128]
                                    .rearrange("p (b f) -> p b f", f=HID),
                                )
                            if rem:
                                nc.sync.dma_start(
                                    out=sh2_d[QOFF[q] + full * 128:QOFF[q + 1], :],
                                    in_=stage[0:rem,
                                              full * 128:full * 128 + HID],
                                )
                            nc.gpsimd.collective_compute(
                                "AllGather", mybir.AluOpType.bypass,
                                replica_groups=rg,
                                ins=[sh2_d[QOFF[q]:QOFF[q + 1], :].opt()],
                                outs=[tab2_q[q][:].opt()],
                            )

                # ---- pass 2 + heads ----
                with (
                    tc.tile_pool(name="hd", bufs=2) as hp,
                    tc.tile_pool(name="hst", bufs=2) as hsp,
                    tc.tile_pool(name="hps", bufs=2, space="PSUM") as hpp,
                ):
                    def out2(p):
                        pc = p * 256
                        w = min(SH, pc + 256) - pc
                        zt = hp.tile([128, 256], b16, tag="zt")
                        nc.vector.tensor_tensor(
                            out=zt[:], in0=z_acc[:, pc:pc + 256],
                            in1=disrep[:, pc:pc + 256],
                            op=mybir.AluOpType.mult)
                        hps = hpp.tile([OUT, 512], f32, space="PSUM",
                                       tag="hps", name="hps")
                        nc.tensor.matmul(out=hps[:, 0:256], lhsT=w2_t[:],
                                         rhs=zt[:], start=True, stop=True)
                        nc.tensor.matmul(out=hps[:, 256:512], lhsT=w3_t[:],
                                         rhs=zt[:], start=True, stop=True)
                        smu = hsp.tile([OUT, 256], f32, tag="smu")
                        sls = hsp.tile([OUT, 256], f32, tag="sls")
                        nc.scalar.activation(
                            out=smu[:], in_=hps[:, 0:256],
                            func=mybir.ActivationFunctionType.Identity,
                            bias=b2_c)
                        nc.scalar.activation(
                            out=sls[:], in_=hps[:, 256:512],
                            func=mybir.ActivationFunctionType.Identity,
                            bias=b3_c)
                        nc.sync.dma_start(out=omu[:, pc:pc + w],
                                          in_=smu[:, 0:w])
                        nc.sync.dma_start(out=ols[:, pc:pc + w],
                                          in_=sls[:, 0:w])

                    agg_pass(tab2_q, sh2_d, out2, (mp1, pp1, zp1, op1, ip1))

    nc.compile()
    return nc


def _preprocess(x, edge_index, W1, b1, gamma, beta, W2, b2, W3, b3):
    src = np.asarray(edge_index[0], dtype=np.int64)
    dst = np.asarray(edge_index[1], dtype=np.int64)
    deg = 1.0 + np.bincount(dst, minlength=N).astype(np.float64)
    dis = (1.0 / np.sqrt(deg)).astype(np.float32)

    core = dst // SH
    dl = dst % SH
    pair = dl // 256
    dloc = dl % 256

    sc = src // SH
    sloc = src % SH
    q = np.digitize(sloc, QOFF[1:4])            # 0..3
    qrows = np.asarray(QROWS)[q]
    rel = (sc * qrows + sloc - np.asarray(QOFF[:4])[q]).astype(np.int64)

    # group key: (core, range, pair)  -- range-major slot space
    gkey = (core * NRANGE + q) * NPAIR + pair
    ngroups = NCORES * NRANGE * NPAIR
    cnt = np.bincount(gkey, minlength=ngroups)
    order = np.argsort(gkey, kind="stable")
    start = np.zeros(ngroups + 1, np.int64)
    np.cumsum(cnt, out=start[1:])
    rank = np.arange(len(gkey)) - start[gkey[order]]
    cntm = cnt.reshape(NCORES, NRANGE, NPAIR)
    gt = ((cntm.max(axis=0) + 127) // 128)      # tiles[range][pair]
    tot_tiles = int(gt.sum())
    call_off = np.zeros((NRANGE, NPAIR), np.int64)
    flat = gt.reshape(-1)
    call_off.reshape(-1)[1:] = np.cumsum(flat)[:-1]
    tmax = int(gt.max())
    tmaxc = 0
    for rr in range(NRANGE):
        for cg in range(NCG):
            tmaxc = max(tmaxc, int(gt[rr][cg * GP:(cg + 1) * GP].sum()))

    # slot for each edge: group's tile base * 128 + rank
    gkey_o = gkey[order]
    r_o = (gkey_o // NPAIR) % NRANGE
    p_o = gkey_o % NPAIR
    slot = call_off[r_o, p_o] * 128 + rank
    core_o = gkey_o // (NRANGE * NPAIR)

    plan = dict(
        tiles=[[int(gt[rr][pp]) for pp in range(NPAIR)]
               for rr in range(NRANGE)],
        call_off=[[int(call_off[rr][pp]) for pp in range(NPAIR)]
                  for rr in range(NRANGE)],
        tot_tiles=tot_tiles,
        tmax=tmax,
        tmaxc=tmaxc,
    )

    rel_o = rel[order]
    dloc_o = dloc[order]

    in_maps = []
    tot_slots = tot_tiles * 128
    iota = np.tile(np.arange(128, dtype=np.float32), (128, 1))
    pcol = np.arange(128, dtype=np.float32).reshape(128, 1)
    metab_a = np.concatenate([iota, pcol], axis=1).astype(bf16)
    iotar_a = np.tile(np.arange(256, dtype=np.float32),
                      (128, tmaxc)).astype(bf16)
    for c in range(NCORES):
        m = core_o == c
        idx_flat = np.zeros(tot_slots, np.int16)
        idx_flat[slot[m]] = rel_o[m].astype(np.int16)

        arr = idx_flat.reshape(tot_slots // 16, 16).T.copy()  # [16, S/16]
        idx16_a = np.tile(arr, (8, 1))

        dloc_a = np.full((128, tot_tiles), 512.0, np.float32)
        sl = slot[m]
        dloc_a[sl % 128, sl // 128] = dloc_o[m]

        base = c * SH
        disn = np.zeros(NB * 128, np.float32)
        disn[:SH] = dis[base:base + SH]
        disrep_a = np.tile(disn[None, :], (128, 1)).astype(bf16)
        discol_a = disn.reshape(NB, 128).T.copy()

        crow_a = np.zeros((128, 4), np.float32)
        crow_a[:, 0] = np.asarray(gamma, np.float32)
        crow_a[:, 1] = np.asarray(beta, np.float32)
        crow_a[:OUT, 2] = np.asarray(b2, np.float32)
        crow_a[:OUT, 3] = np.asarray(b3, np.float32)

        xs = np.asarray(x[base:base + SH], np.float32)
        xT_a = np.zeros((IN, NB * 128), np.float32)
        xT_a[:, :SH] = xs.T
        in_maps.append(dict(
            xT=xT_a.astype(bf16),
            w1=np.asarray(W1, np.float32).astype(bf16),
            w2=np.asarray(W2, np.float32).astype(bf16),
            w3=np.asarray(W3, np.float32).astype(bf16),
            crow=crow_a,
            disrep=disrep_a,
            discol=discol_a,
            dloct=dloc_a.astype(bf16),
            ndloct=(-dloc_a).astype(bf16),
            iotar=iotar_a,
            idx16=idx16_a,
            metab=metab_a,
        ))
    return in_maps, plan


_NC_CACHE = {}


def kernel(**inputs):
    in_maps, plan = _preprocess(**inputs)
    key = (plan["tot_tiles"], plan["tmax"], plan["tmaxc"])
    if key not in _NC_CACHE:
        _NC_CACHE[key] = _build_nc(plan)
    nc = _NC_CACHE[key]
    res = run_bass_kernel_spmd(nc, in_maps, core_ids=list(range(NCORES)))
    xm = np.concatenate([res.results[c]["omu"].T for c in range(NCORES)], axis=0)
    x_ = np.concatenate([res.results[c]["ols"].T for c in range(NCORES)], axis=0)
    return xm.astype(np.float32), x_.astype(np.float32)
